# revision 22
# baseline (speedup 1.0000x reference)
"""Self-contained Trainium2 Bass kernel for nn_LunarCausalAttention.

Sharding: 8 cores = 2 batches x 4 head-blocks (4 heads each). Params sliced
per core host-side; per-core partial outputs (over head-blocks) summed on
host during the gather (plus bo).

All matmuls use PE row base 0 (mixing row bases within one PSUM bank is a
fatal HW hazard). Odd-head operand halves are DMA-shifted to partitions
0-63; kp is folded into an on-device effective weight W_eff = Wpc^T @ pq^T.
"""

import math

import ml_dtypes
import numpy as np

import concourse.bacc as bacc
import concourse.bass as bass
import concourse.mybir as mybir
import concourse.tile as tile

EMBED = 1024
D = 64
PLEN = 32
NTOK = 2048
BSZ = 2
SCALING = D ** -0.5
BETA = math.log(2.0)

NH = 4           # heads per core
C = 128          # chunk (token tile)
NCH = NTOK // C  # 16 chunks
F32 = mybir.dt.float32
BF16 = mybir.dt.bfloat16
AX = mybir.AxisListType
AF = mybir.ActivationFunctionType


def _bcast(ap_obj, dim_count, at=1):
    """Insert a stride-0 dim of size dim_count into an AP at free position."""
    pat = [list(p) for p in ap_obj.ap]
    pat.insert(at, [0, dim_count])
    return bass.AP(tensor=ap_obj.tensor, offset=ap_obj.offset, ap=pat)


def build_nc(stage=99):
    nc = bacc.Bacc("TRN2", target_bir_lowering=False, debug=False,
                   num_devices=8)

    xT_d = nc.dram_tensor("xT", [EMBED, NTOK], BF16, kind="ExternalInput")
    pxT_d = nc.dram_tensor("pxT", [EMBED, PLEN], BF16, kind="ExternalInput")
    wqc_d = nc.dram_tensor("wqcT", [EMBED, 4 * C], BF16, kind="ExternalInput")
    bqc_d = nc.dram_tensor("bqc", [4 * C], F32, kind="ExternalInput")
    wpq_d = nc.dram_tensor("wpqT", [EMBED, 2 * C], BF16, kind="ExternalInput")
    bpq_d = nc.dram_tensor("bpq", [2 * C], F32, kind="ExternalInput")
    wpc_d = nc.dram_tensor("wpcR", [D, NH, 8, 128], BF16, kind="ExternalInput")
    bpc_d = nc.dram_tensor("bpc0", [D, NH], BF16, kind="ExternalInput")
    wo_d = nc.dram_tensor("woT", [NH * D, EMBED], BF16, kind="ExternalInput")
    rlen_d = nc.dram_tensor("rlen", [C, NCH], F32, kind="ExternalInput")
    mask_d = nc.dram_tensor("mask", [C, C], F32, kind="ExternalInput")
    id64p_d = nc.dram_tensor("id64p", [128, 2, D], BF16, kind="ExternalInput")
    id128_d = nc.dram_tensor("id128", [128, 128], BF16, kind="ExternalInput")
    out_d = nc.dram_tensor("out", [NTOK, EMBED], F32, kind="ExternalOutput")

    with tile.TileContext(nc) as tc:
        with (
            tc.tile_pool(name="big", bufs=1) as big,
            tc.tile_pool(name="wstr", bufs=4) as wstr,
            tc.tile_pool(name="work", bufs=2) as work,
            tc.tile_pool(name="outp", bufs=2) as outp,
            tc.tile_pool(name="psp", bufs=1, space="PSUM") as psp,
        ):
            # ---- persistent loads ----
            xT = big.tile([128, 8, NTOK], BF16)
            nc.sync.dma_start(out=xT, in_=xT_d.rearrange("(k p) n -> p k n", p=128))
            pxT = big.tile([128, 8, PLEN], BF16)
            nc.sync.dma_start(out=pxT, in_=pxT_d.rearrange("(k p) n -> p k n", p=128))
            bpc0 = big.tile([D, NH], BF16)
            nc.sync.dma_start(out=bpc0, in_=bpc_d.ap())
            wo = big.tile([128, 2, EMBED], BF16)
            nc.sync.dma_start(out=wo, in_=wo_d.rearrange("(k p) o -> p k o", p=128))
            bqc = big.tile([128, 4], F32)
            nc.sync.dma_start(out=bqc, in_=bqc_d.rearrange("(m p) -> p m", p=128))
            bpq = big.tile([128, 2], F32)
            nc.sync.dma_start(out=bpq, in_=bpq_d.rearrange("(m p) -> p m", p=128))
            rlen = big.tile([C, NCH], F32)
            nc.sync.dma_start(out=rlen, in_=rlen_d.ap())
            mask = big.tile([C, C], F32)
            nc.sync.dma_start(out=mask, in_=mask_d.ap())
            id64p = big.tile([128, 2, D], BF16)
            nc.sync.dma_start(out=id64p, in_=id64p_d.ap())
            id128 = big.tile([128, 128], BF16)
            nc.sync.dma_start(out=id128, in_=id128_d.ap())
            ones1 = big.tile([1, 512], BF16)
            nc.vector.memset(ones1, 1.0)

            lin = big.tile([128, 4, NTOK], BF16)      # q(0,1) kv(2,3)
            lin0 = big.tile([D, 4, NTOK], BF16)       # odd halves at base 0
            kvtok = big.tile([128, NCH, NH, D], BF16)
            weff = big.tile([128, 8, NH, PLEN], BF16)
            pq_sb = big.tile([128, 2, PLEN], BF16)
            pq0 = big.tile([D, NH, PLEN], BF16)
            be_sb = big.tile([1, NH * PLEN], BF16)
            S1 = big.tile([D, NH, PLEN], F32)
            S2 = big.tile([128, D], F32)
            S1b = big.tile([D, NH, PLEN], BF16)
            S2b = big.tile([128, D], BF16)

            def q_at0(h, tok):
                g, half = h // 2, h % 2
                return (lin0[:, g, tok] if half else lin[0:D, g, tok])

            def kv_at0(h, tok):
                g, half = h // 2, h % 2
                return (lin0[:, 2 + g, tok] if half else lin[0:D, 2 + g, tok])

            # ---- pq linear (channel-major) ----
            wpq_r = wpq_d.rearrange("(k p) m -> p k m", p=128)
            for m in range(2):
                ps = psp.tile([128, PLEN], F32, tag="pE")
                for k in range(8):
                    wpqt = wstr.tile([128, 128], BF16, tag="wt", name="wpqt")
                    nc.sync.dma_start(out=wpqt,
                                      in_=wpq_r[:, k, m * 128:(m + 1) * 128])
                    nc.tensor.matmul(ps, lhsT=wpqt, rhs=pxT[:, k, :],
                                     start=(k == 0), stop=(k == 7))
                nc.scalar.activation(out=pq_sb[:, m, :], in_=ps, func=AF.Identity,
                                     bias=bpq[:, m:m + 1], scale=1.0)

            # pq0: per-head pq at partitions 0-63 (SBUF->SBUF DMA shift)
            for h in range(NH):
                g, half = h // 2, h % 2
                nc.sync.dma_start(out=pq0[:, h, :],
                                  in_=pq_sb[64 * half:64 * half + D, g, :])

            # bias_eff[h, p] = bpc_h . pq_h[:, p]  (exactness for nonzero bpc)
            be_ps = psp.tile([1, NH, PLEN], F32, tag="pE")
            for h in range(NH):
                nc.tensor.matmul(be_ps[:, h, :], lhsT=bpc0[:, h:h + 1],
                                 rhs=pq0[:, h, :], start=True, stop=True)
            nc.vector.tensor_copy(be_sb, be_ps.rearrange("p h w -> p (h w)"))

            # ---- W_eff[e, (h,p)] = sum_d Wpc[(h,d), e] * pq[h, p, d] ----
            for k in range(8):
                wpck = wstr.tile([D, NH, 128], BF16, tag="wpck", bufs=2)
                nc.sync.dma_start(out=wpck, in_=wpc_d[:, :, k, :])
                ps = psp.tile([128, NH, PLEN], F32, tag="pE")
                for h in range(NH):
                    nc.tensor.matmul(ps[:, h, :], lhsT=wpck[:, h, :],
                                     rhs=pq0[:, h, :], start=True, stop=True)
                nc.scalar.copy(weff[:, k, :, :], ps)

            # ---- q/kv linears (weights streamed; 4 psum banks) ----
            wqc_r = wqc_d.rearrange("(k p) m -> p k m", p=128)
            for m in range(4):
                pss = [psp.tile([128, 512], F32, tag=t, name=f"lin_{t}")
                       for t in ("pA", "pB", "pC", "pD")]
                for k in range(8):
                    wt = wstr.tile([128, 128], BF16, tag="wt")
                    nc.sync.dma_start(out=wt,
                                      in_=wqc_r[:, k, m * 128:(m + 1) * 128])
                    for nt in range(4):
                        nc.tensor.matmul(pss[nt],
                                         lhsT=wt,
                                         rhs=xT[:, k, nt * 512:(nt + 1) * 512],
                                         start=(k == 0), stop=(k == 7))
                for nt in range(4):
                    nc.scalar.activation(
                        out=lin[:, m, nt * 512:(nt + 1) * 512], in_=pss[nt],
                        func=AF.Identity, bias=bqc[:, m:m + 1], scale=1.0)

            # ---- lin0: odd halves shifted to partitions 0-63 ----
            for j in range(4):
                nc.sync.dma_start(out=lin0[:, j, :], in_=lin[D:128, j, :])

            # ---- pattn + softplus pre-phase, channel-major [ (h,p), tok ] ----
            z_cm = big.tile([128, NTOK], BF16)
            for nt in range(4 if stage >= 3 else 0):
                sl = slice(nt * 512, (nt + 1) * 512)
                pps = psp.tile([128, 512], F32, tag="pH", name="pat_ps")
                for k in range(8):
                    nc.tensor.matmul(
                        pps, lhsT=weff[:, k, :, :].rearrange("p h w -> p (h w)"),
                        rhs=xT[:, k, sl], start=(k == 0), stop=False)
                nc.tensor.matmul(pps, lhsT=be_sb, rhs=ones1,
                                 start=False, stop=True)
                nc.scalar.activation(out=z_cm[:, sl], in_=pps, func=AF.Exp,
                                     scale=BETA)
            if stage >= 3:
                nc.scalar.activation(out=z_cm, in_=z_cm, func=AF.Ln, bias=1.0)

            # ---- kv transposes to token-major (padded-identity trick) ----
            for c in range(NCH if stage >= 2 else 0):
                tok = slice(c * C, (c + 1) * C)
                ps = psp.tile([128, NH, D], F32, tag="pH")
                for h in range(NH):
                    g, half = h // 2, h % 2
                    nc.tensor.matmul(ps[:, h, :], lhsT=lin[:, 2 + g, tok],
                                     rhs=id64p[:, half, :],
                                     start=True, stop=True)
                nc.vector.tensor_copy(kvtok[:, c], ps)

            # ---- chunk scan loop ----
            for c in range(NCH if stage >= 3 else 0):
                tok = slice(c * C, (c + 1) * C)
                # z token-major for this chunk (single PE transpose)
                ztp = psp.tile([128, 128], BF16, tag="pF", name="ztp")
                nc.tensor.matmul(ztp, lhsT=z_cm[:, tok], rhs=id128,
                                 start=True, stop=True, is_transpose=True)
                z_sb = work.tile([128, 128], BF16, tag="z_sb")
                nc.vector.tensor_copy(z_sb, ztp)

                pd = psp.tile([128, 384], F32, tag="pD")
                # M1 + mask (all heads at row base 0)
                m1 = psp.tile([128, NH, C], F32, tag="pA")
                for h in range(NH):
                    nc.tensor.matmul(m1[:, h, :], lhsT=kv_at0(h, tok),
                                     rhs=q_at0(h, tok), start=True, stop=True)
                m1m = work.tile([128, NH, C], BF16, tag="m1m")
                nc.vector.tensor_mul(m1m, m1, _bcast(mask, NH))

                # out1 = intra + inter  -> pd[:, 0:128]
                for h in range(NH):
                    o1 = pd[:, h * PLEN:(h + 1) * PLEN]
                    nc.tensor.matmul(o1, lhsT=m1m[:, h, :],
                                     rhs=z_sb[:, h * PLEN:(h + 1) * PLEN],
                                     start=True, stop=(c == 0))
                    if c > 0:
                        nc.tensor.matmul(o1, lhsT=q_at0(h, tok),
                                         rhs=S1b[:, h, :],
                                         start=False, stop=True)

                # dS1 / dS2 (pG) + state updates (in place)
                pg = psp.tile([128, 192], F32, tag="pG")
                dS1 = pg[0:D, 64:192].rearrange("p (h w) -> p h w", w=PLEN)
                for h in range(NH):
                    nc.tensor.matmul(dS1[:, h, :], lhsT=kvtok[:, c, h, :],
                                     rhs=z_sb[:, h * PLEN:(h + 1) * PLEN],
                                     start=True, stop=True)
                    nc.tensor.matmul(pg[32 * h:32 * h + 32, 0:D],
                                     lhsT=z_sb[:, h * PLEN:(h + 1) * PLEN],
                                     rhs=kvtok[:, c, h, :],
                                     start=True, stop=True,
                                     tile_position=(0, 32 * h))
                def update_states():
                    if c == 0:
                        nc.vector.tensor_copy(S1, dS1)
                        nc.vector.tensor_copy(S2, pg[:, 0:D])
                    else:
                        nc.vector.tensor_add(S1, dS1, S1)
                        nc.vector.tensor_add(S2, pg[:, 0:D], S2)
                    nc.vector.tensor_copy(S1b, S1)
                    nc.vector.tensor_copy(S2b, S2)

                # softmax over plen (batched across heads) + rlen scales
                t0 = work.tile([128, NH * PLEN], F32, tag="t0")
                nc.vector.tensor_scalar_mul(t0, pd[:, 0:128], rlen[:, c:c + 1])
                nmx = work.tile([128, 1], F32, tag="nmx")
                nc.vector.reduce_max(nmx, t0, axis=AX.X, negate=True)
                e_sb = work.tile([128, NH, PLEN], F32, tag="e_sb")
                nc.scalar.activation(out=e_sb,
                                     in_=t0.rearrange("p (h w) -> p h w", h=NH),
                                     func=AF.Exp, bias=nmx, scale=1.0)
                ssum = work.tile([128, NH], F32, tag="ssum")
                nc.vector.reduce_sum(ssum, e_sb, axis=AX.X)
                rs = work.tile([128, NH], F32, tag="rs")
                nc.vector.reciprocal(rs, ssum)
                rs2 = work.tile([128, NH], F32, tag="rs2")
                nc.vector.tensor_scalar_mul(rs2, rs, rlen[:, c:c + 1])
                aw = work.tile([128, NH, PLEN], BF16, tag="aw")
                nc.vector.tensor_mul(aw, e_sb, _bcast(rs2, PLEN, at=2))

                if stage < 5:
                    update_states()
                    continue
                # aw transpose -> awT stacked [(h,p), tok] (rows 32h per head)
                awp = psp.tile([128, 128], BF16, tag="pF", name="awp")
                nc.tensor.matmul(awp, lhsT=aw.rearrange("p h w -> p (h w)"),
                                 rhs=id128, start=True, stop=True,
                                 is_transpose=True)
                awT = work.tile([128, 128], BF16, tag="awT")
                nc.vector.tensor_copy(awT, awp)

                # M2 + mask: row base 32h, alternating psum banks (pB/pE)
                m2m = []
                for h in range(NH):
                    p0 = 32 * h
                    m2h = psp.tile([128, 128], F32, tag=("pB" if h % 2 == 0
                                                         else "pE"),
                                   name=f"m2h{h % 2}")
                    nc.tensor.matmul(m2h, lhsT=z_cm[p0:p0 + 32, tok],
                                     rhs=awT[p0:p0 + 32, :],
                                     start=True, stop=True,
                                     tile_position=(p0, 0))
                    mm = work.tile([128, 128], BF16, tag=f"m2m{h % 2}")
                    nc.vector.tensor_mul(mm, m2h, mask)
                    m2m.append(mm)

                if stage < 6:
                    update_states()
                    continue
                # out2T intra -> pd[:, 128:384]
                for h in range(NH):
                    g, half = h // 2, h % 2
                    nc.tensor.matmul(
                        pd[64 * half:64 * half + 64,
                           128 + g * C:128 + (g + 1) * C],
                        lhsT=kvtok[:, c, h, :], rhs=m2m[h],
                        start=True, stop=True, tile_position=(0, 64 * half))
                attnT = work.tile([128, 2, C], BF16, tag="attnT")
                nc.scalar.copy(attnT,
                               pd[:, 128:384].rearrange("p (g w) -> p g w", w=C))
                # out2T inter: row base 32h, alternating banks; add into attnT
                if c > 0:
                    for h in range(NH):
                        g, half = h // 2, h % 2
                        p0 = 32 * h
                        o2h = psp.tile([128, 128], F32,
                                       tag=("pB" if h % 2 == 0 else "pE"),
                                       name=f"o2h{h % 2}")
                        nc.tensor.matmul(o2h[64 * half:64 * half + 64, :],
                                         lhsT=S2b[p0:p0 + 32, :],
                                         rhs=awT[p0:p0 + 32, :],
                                         start=True, stop=True,
                                         tile_position=(p0, 64 * half))
                        nc.vector.tensor_add(
                            attnT[64 * half:64 * half + 64, g, :],
                            o2h[64 * half:64 * half + 64, :],
                            attnT[64 * half:64 * half + 64, g, :])

                update_states()

                # final projection; bo added on host during gather
                for nh in range(2):
                    osl = slice(nh * 512, (nh + 1) * 512)
                    fp = psp.tile([128, 512], F32, tag="pC")
                    for kt in range(2):
                        nc.tensor.matmul(fp, lhsT=attnT[:, kt, :],
                                         rhs=wo[:, kt, osl],
                                         start=(kt == 0), stop=(kt == 1))
                    ob = outp.tile([128, 512], F32, tag="ob")
                    if nh == 0:
                        nc.vector.tensor_copy(ob, fp)
                    else:
                        nc.scalar.copy(ob, fp)
                    nc.sync.dma_start(out=out_d[tok, osl], in_=ob)

    nc.compile()
    return nc


_NC = None


def get_nc():
    global _NC
    if _NC is None:
        _NC = build_nc()
    return _NC


def make_in_maps(query, pquery, Wpq, bpq, Wq, bq, Wpc, bpc, Wc, bc, Wo, bo):
    query = np.asarray(query, np.float32)
    pquery = np.asarray(pquery, np.float32)
    Wpq, Wq, Wpc, Wc, Wo = (np.asarray(w, np.float32)
                            for w in (Wpq, Wq, Wpc, Wc, Wo))
    bpq_, bq_, bpc_, bc_ = (np.asarray(v, np.float32)
                            for v in (bpq, bq, bpc, bc))
    n_idx = np.arange(NTOK, dtype=np.float64)
    rlen = (1.0 / ((n_idx + 1.0) * BETA)).astype(np.float32)
    rlen = np.ascontiguousarray(rlen.reshape(NCH, C).T)          # [C, NCH]
    mask = np.triu(np.ones((C, C), np.float32))                  # keep j <= i
    id64p = np.zeros((128, 2, D), np.float32)
    id64p[np.arange(64), 0, np.arange(64)] = 1.0
    id64p[np.arange(64, 128), 1, np.arange(64)] = 1.0
    id128 = np.eye(128, dtype=np.float32)

    in_maps = []
    for core in range(8):
        b, hb = core // 4, core % 4
        ch = slice(hb * NH * D, (hb + 1) * NH * D)
        wqcT = np.concatenate([SCALING * Wq[ch], Wc[ch]], axis=0).T
        bqc = np.concatenate([SCALING * bq_[ch], bc_[ch]])
        wpcR = np.ascontiguousarray(
            Wpc[ch].reshape(NH, D, 8, 128).transpose(1, 0, 2, 3))
        bf = ml_dtypes.bfloat16
        in_maps.append({
            "xT": np.ascontiguousarray(query[:, b, :].T).astype(bf),
            "pxT": np.ascontiguousarray(pquery[:, b, :].T).astype(bf),
            "wqcT": np.ascontiguousarray(wqcT).astype(bf),
            "bqc": np.ascontiguousarray(bqc),
            "wpqT": np.ascontiguousarray((SCALING * Wpq[ch]).T).astype(bf),
            "bpq": np.ascontiguousarray(SCALING * bpq_[ch]),
            "wpcR": wpcR.astype(bf),
            "bpc0": np.ascontiguousarray(bpc_[ch].reshape(NH, D).T).astype(bf),
            "woT": np.ascontiguousarray(Wo[:, ch].T).astype(bf),
            "rlen": rlen, "mask": mask,
            "id64p": id64p.astype(bf), "id128": id128.astype(bf),
        })
    return in_maps


def kernel(**inputs):
    from concourse.bass_utils import run_bass_kernel_spmd
    nc = get_nc()
    in_maps = make_in_maps(**inputs)
    res = run_bass_kernel_spmd(nc, in_maps, core_ids=list(range(8)))
    bo = np.asarray(inputs["bo"], np.float32)
    out = np.zeros((NTOK, BSZ, EMBED), np.float32)
    for b in range(BSZ):
        acc = res.results[4 * b]["out"].astype(np.float32).copy()
        for i in range(1, 4):
            acc += res.results[4 * b + i]["out"]
        out[:, b, :] = acc + bo
    return out
